# revision 1
# baseline (speedup 1.0000x reference)
"""GRU decoder kernel for Trainium2 (Bass/Tile), data-parallel over 8 NeuronCores.

Problem: nn_Decoder (B=512, T=128, D=256, H=1024), PyTorch GRUCell semantics:
    gi = x @ W_ih.T + b_ih ; gh = h @ W_hh.T + b_hh
    r = sig(gi_r + gh_r); z = sig(gi_z + gh_z); n = tanh(gi_n + r*gh_n)
    h' = (1-z)*n + z*h ; y = x + h' @ W_tp.T + b_tp ; x' = y   (x0 = gt[:,0,:])

Sharding: batch 512 -> 64 per core, weights replicated. All matmuls in f32r
(full-rate fp32-reduced, ~13-14 mantissa bits), fp32 PSUM accumulation,
fp32 state/gate arithmetic. Per step, per core (batch on PSUM partitions
M=64, weights streamed as the moving operand):
    p_r  [64,1024] = b_r 1s-row + 8 hT chunks + 2 xT chunks   (11 K-chunks)
    p_hn [64,1024] = b_hn + 8 hT chunks
    p_in [64,1024] = b_in + 2 xT chunks
    p_z  [64,1024] = b_z + 8 + 2
    n-path: t1 = r*p_hn; t2 = t1 + p_in; n = tanh(t2)  (overlaps p_z matmuls)
    h' = n + z*(h - n); hT via 8 PE transposes -> one f32r copy
    p_y [64,256] = b_tp + 8 hT' chunks ; y = x + p_y ; xT via 2 PE transposes
"""
import numpy as np

B, T, D, H = 512, 128, 256, 1024
NCORES = 8
BL = B // NCORES  # 64 batch rows per core
H3 = 3 * H

_CACHE = {}


def _build(nsteps, loop_reps=None):
    import concourse.bass as bass
    import concourse.mybir as mybir
    import concourse.tile as tile
    from concourse import bacc
    from concourse.masks import make_identity

    F32 = mybir.dt.float32
    F32R = mybir.dt.float32r
    AF = mybir.ActivationFunctionType

    nc = bacc.Bacc(None, target_bir_lowering=False)

    # --- DRAM I/O (per core). Host pre-transposes weights/state.
    x0_d = nc.dram_tensor("x0", [BL, D], F32, kind="ExternalInput")
    x0T_d = nc.dram_tensor("x0T", [D, BL], F32R, kind="ExternalInput")
    h0_d = nc.dram_tensor("h0", [BL, H], F32, kind="ExternalInput")
    h0T_d = nc.dram_tensor("h0T", [H, BL], F32R, kind="ExternalInput")
    wih_d = nc.dram_tensor("W_ihT", [D, H3], F32R, kind="ExternalInput")   # x-side
    whh_d = nc.dram_tensor("W_hhT", [H, H3], F32R, kind="ExternalInput")   # h-side
    wtp_d = nc.dram_tensor("W_tpT", [H, D], F32R, kind="ExternalInput")    # y-side
    # bias rows (1 x N), ones row handled via these directly
    brz_d = nc.dram_tensor("brz", [1, 2 * H], F32R, kind="ExternalInput")  # b_ih+b_hh for r,z
    bin_d = nc.dram_tensor("bin", [1, H], F32R, kind="ExternalInput")      # b_ih n-part
    bhn_d = nc.dram_tensor("bhn", [1, H], F32R, kind="ExternalInput")      # b_hh n-part
    btp_d = nc.dram_tensor("btp", [1, D], F32R, kind="ExternalInput")
    ones_d = nc.dram_tensor("ones", [1, BL], F32R, kind="ExternalInput")
    Y_d = nc.dram_tensor("Y", [BL, T, D], F32, kind="ExternalOutput")

    KH = H // 128   # 8 h chunks
    KD = D // 128   # 2 x chunks

    with tile.TileContext(nc) as tc:
        with (
            tc.tile_pool(name="wpool", bufs=1) as wpool,
            tc.tile_pool(name="state", bufs=2) as state,
            tc.tile_pool(name="gates", bufs=1) as gates,
            tc.tile_pool(name="ypool", bufs=(2 if loop_reps else 3)) as ypool,
            tc.tile_pool(name="ps_rz", bufs=1, space="PSUM") as ps_rz,
            tc.tile_pool(name="ps_hn", bufs=1, space="PSUM") as ps_hn,
            tc.tile_pool(name="ps_iy", bufs=1, space="PSUM") as ps_iy,
            tc.tile_pool(name="ps_tr", bufs=1, space="PSUM") as ps_tr,
        ):
            # --- load weights (resident in SBUF for entire kernel)
            wih = wpool.tile([128, KD, H3], F32R)   # [128, 2, 3072]
            whh = wpool.tile([128, KH, H3], F32R)   # [128, 8, 3072]
            wtp = wpool.tile([128, KH, D], F32R)    # [128, 8, 256]
            for c in range(KD):
                nc.sync.dma_start(out=wih[:, c, :], in_=wih_d[c * 128:(c + 1) * 128, :])
            for c in range(KH):
                nc.sync.dma_start(out=whh[:, c, :], in_=whh_d[c * 128:(c + 1) * 128, :])
                nc.sync.dma_start(out=wtp[:, c, :], in_=wtp_d[c * 128:(c + 1) * 128, :])
            brz = wpool.tile([1, 2 * H], F32R)
            bin_ = wpool.tile([1, H], F32R)
            bhn = wpool.tile([1, H], F32R)
            btp = wpool.tile([1, D], F32R)
            ones = wpool.tile([1, BL], F32R)
            nc.sync.dma_start(out=brz, in_=brz_d[:, :])
            nc.sync.dma_start(out=bin_, in_=bin_d[:, :])
            nc.sync.dma_start(out=bhn, in_=bhn_d[:, :])
            nc.sync.dma_start(out=btp, in_=btp_d[:, :])
            nc.sync.dma_start(out=ones, in_=ones_d[:, :])
            ident = wpool.tile([128, 128], F32)
            make_identity(nc, ident)

            # --- initial state (h split into two 512-wide halves for pipelining)
            HB = H // 2            # 512
            CH = KH // 2           # 4 chunks per half
            x_nat = state.tile([BL, D], F32, tag="x_nat")
            h_a = state.tile([BL, HB], F32, tag="h_a")
            h_b = state.tile([BL, HB], F32, tag="h_b")
            xT = state.tile([128, KD * BL], F32R, tag="xT")     # [128, 2*64]
            hT_a = state.tile([128, CH * BL], F32R, tag="hT_a")  # chunks 0..3
            hT_b = state.tile([128, CH * BL], F32R, tag="hT_b")  # chunks 4..7
            nc.sync.dma_start(out=x_nat, in_=x0_d[:, :])
            nc.sync.dma_start(out=h_a, in_=h0_d[:, 0:HB])
            nc.sync.dma_start(out=h_b, in_=h0_d[:, HB:H])
            for c in range(KD):
                nc.sync.dma_start(out=xT[:, c * BL:(c + 1) * BL],
                                  in_=x0T_d[c * 128:(c + 1) * 128, :])
            for c in range(CH):
                nc.sync.dma_start(out=hT_a[:, c * BL:(c + 1) * BL],
                                  in_=h0T_d[c * 128:(c + 1) * 128, :])
                nc.sync.dma_start(out=hT_b[:, c * BL:(c + 1) * BL],
                                  in_=h0T_d[(CH + c) * 128:(CH + c + 1) * 128, :])

            from contextlib import nullcontext
            loop_cm = tc.For_i(0, loop_reps, 1) if loop_reps else nullcontext()
            with loop_cm:
              for t in range(nsteps):
                hT_pair = (hT_a, hT_b)

                def hT_chunk(c):
                    tile_, cc = (hT_pair[0], c) if c < CH else (hT_pair[1], c - CH)
                    return tile_[:, cc * BL:(cc + 1) * BL]

                def acc_half(psum, gc0, use_h, use_x, bias_ap, bias_c0):
                    """Accumulate one 512-wide region: bias row, h chunks, x chunks."""
                    chunks = [(ones[:, :], bias_ap[:, bias_c0:bias_c0 + 512])]
                    if use_h:
                        for c in range(KH):
                            chunks.append((hT_chunk(c), whh[:, c, gc0:gc0 + 512]))
                    if use_x:
                        for c in range(KD):
                            chunks.append((xT[:, c * BL:(c + 1) * BL],
                                           wih[:, c, gc0:gc0 + 512]))
                    for i, (lhsT, rhs) in enumerate(chunks):
                        nc.tensor.matmul(psum, lhsT, rhs,
                                         start=(i == 0), stop=(i == len(chunks) - 1))

                # ---- gate matmuls, region-major: r0 r1 hn0 hn1 in0 in1 z0 z1
                p_r, p_hn, p_in, p_z = [], [], [], []
                for k in range(2):
                    p = ps_rz.tile([BL, HB], F32, tag=f"rz{k}")
                    acc_half(p, k * HB, True, True, brz, k * HB)
                    p_r.append(p)
                for k in range(2):
                    p = ps_hn.tile([BL, HB], F32, tag=f"hn{k}")
                    acc_half(p, 2 * H + k * HB, True, False, bhn, k * HB)
                    p_hn.append(p)
                for k in range(2):
                    p = ps_iy.tile([BL, HB], F32, tag=f"iy{k}")
                    acc_half(p, 2 * H + k * HB, False, True, bin_, k * HB)
                    p_in.append(p)
                # r sigmoids can start as soon as each r-half completes
                r = []
                for k in range(2):
                    rk = gates.tile([BL, HB], F32, tag=f"r{k}")
                    nc.scalar.activation(rk, p_r[k], AF.Sigmoid)
                    r.append(rk)
                for k in range(2):
                    p = ps_rz.tile([BL, HB], F32, tag=f"rz{k}")
                    acc_half(p, H + k * HB, True, True, brz, H + k * HB)
                    p_z.append(p)

                # ---- n-path per half: n = tanh(i_n + r*h_n); d = h - n
                n, d = [], []
                for k in range(2):
                    t1 = gates.tile([BL, HB], F32, tag=f"t1{k}")
                    nc.vector.tensor_mul(t1, r[k], p_hn[k])
                    t2 = gates.tile([BL, HB], F32, tag=f"t2{k}")
                    nc.vector.tensor_add(t2, t1, p_in[k])
                    nk = gates.tile([BL, HB], F32, tag=f"n{k}")
                    nc.scalar.activation(nk, t2, AF.Tanh)
                    n.append(nk)
                    dk = gates.tile([BL, HB], F32, tag=f"d{k}")
                    nc.vector.tensor_sub(dk, (h_a, h_b)[k], nk)
                    d.append(dk)

                # ---- z sigmoid + lerp + transpose, per half
                h_new, hT_new = [], []
                for k in range(2):
                    zk = gates.tile([BL, HB], F32, tag=f"z{k}")
                    nc.scalar.activation(zk, p_z[k], AF.Sigmoid)
                    uk = gates.tile([BL, HB], F32, tag=f"u{k}")
                    nc.vector.tensor_mul(uk, zk, d[k])
                    hk = state.tile([BL, HB], F32, tag=("h_a", "h_b")[k])
                    nc.vector.tensor_add(hk, n[k], uk)
                    h_new.append(hk)
                    p_tr = ps_tr.tile([128, CH * BL], F32, tag="tr")
                    for c in range(CH):
                        nc.tensor.transpose(
                            p_tr[:, c * BL:(c + 1) * BL],
                            hk[:, c * 128:(c + 1) * 128],
                            ident[:BL, :BL])
                    hTk = state.tile([128, CH * BL], F32R, tag=("hT_a", "hT_b")[k])
                    nc.scalar.copy(hTk, p_tr)
                    hT_new.append(hTk)

                # ---- y = x + h' @ W_tp.T + b_tp
                p_y = ps_iy.tile([BL, D], F32, tag="iy0")
                nc.tensor.matmul(p_y, ones[:, :], btp[:, :], start=True, stop=False)
                for c in range(KH):
                    tile_, cc = (hT_new[0], c) if c < CH else (hT_new[1], c - CH)
                    nc.tensor.matmul(
                        p_y, tile_[:, cc * BL:(cc + 1) * BL], wtp[:, c, :],
                        start=False, stop=(c == KH - 1))
                y = ypool.tile([BL, D], F32, tag="y")
                nc.vector.tensor_add(y, x_nat, p_y)
                nc.sync.dma_start(out=Y_d[:, t, :], in_=y)

                # ---- xT for next step (2 PE transposes)
                p_xt = ps_tr.tile([128, KD * BL], F32, tag="tr")
                for c in range(KD):
                    nc.tensor.transpose(
                        p_xt[:, c * BL:(c + 1) * BL],
                        y[:, c * 128:(c + 1) * 128],
                        ident[:BL, :BL])
                xT_new = state.tile([128, KD * BL], F32R, tag="xT")
                nc.scalar.copy(xT_new, p_xt)

                x_nat, xT = y, xT_new
                h_a, h_b = h_new
                hT_a, hT_b = hT_new

    nc.finalize()
    return nc


def _build_null():
    """Same I/O signature as _build but ~no work: isolates dispatch+transfer
    overhead so test.py can subtract it from steady-state call times."""
    import concourse.mybir as mybir
    import concourse.tile as tile
    from concourse import bacc

    F32 = mybir.dt.float32
    F32R = mybir.dt.float32r
    nc = bacc.Bacc(None, target_bir_lowering=False)
    x0_d = nc.dram_tensor("x0", [BL, D], F32, kind="ExternalInput")
    nc.dram_tensor("x0T", [D, BL], F32R, kind="ExternalInput")
    nc.dram_tensor("h0", [BL, H], F32, kind="ExternalInput")
    nc.dram_tensor("h0T", [H, BL], F32R, kind="ExternalInput")
    nc.dram_tensor("W_ihT", [D, H3], F32R, kind="ExternalInput")
    nc.dram_tensor("W_hhT", [H, H3], F32R, kind="ExternalInput")
    nc.dram_tensor("W_tpT", [H, D], F32R, kind="ExternalInput")
    nc.dram_tensor("brz", [1, 2 * H], F32R, kind="ExternalInput")
    nc.dram_tensor("bin", [1, H], F32R, kind="ExternalInput")
    nc.dram_tensor("bhn", [1, H], F32R, kind="ExternalInput")
    nc.dram_tensor("btp", [1, D], F32R, kind="ExternalInput")
    nc.dram_tensor("ones", [1, BL], F32R, kind="ExternalInput")
    Y_d = nc.dram_tensor("Y", [BL, T, D], F32, kind="ExternalOutput")
    with tile.TileContext(nc) as tc:
        with tc.tile_pool(name="p", bufs=1) as p:
            tmp = p.tile([BL, D], F32)
            nc.sync.dma_start(out=tmp, in_=x0_d[:, :])
            nc.sync.dma_start(out=Y_d[:, 0, :], in_=tmp)
    nc.finalize()
    return nc


def _get_nc(nsteps):
    if nsteps not in _CACHE:
        _CACHE[nsteps] = _build(nsteps)
    return _CACHE[nsteps]


def make_in_maps(h, gt, W_ih, W_hh, b_ih, b_hh, W_tp, b_tp):
    """Host-side prep: slice batch per core, pre-transpose state/weights."""
    f32 = np.float32
    x0 = np.ascontiguousarray(gt[:, 0, :], f32)               # [B, D]
    W_ihT = np.ascontiguousarray(np.asarray(W_ih, f32).T)     # [D, 3H]
    W_hhT = np.ascontiguousarray(np.asarray(W_hh, f32).T)     # [H, 3H]
    W_tpT = np.ascontiguousarray(np.asarray(W_tp, f32).T)     # [H, D]
    b_sum = np.asarray(b_ih, f32) + np.asarray(b_hh, f32)
    brz = np.ascontiguousarray(b_sum[None, :2 * H], f32)
    bin_ = np.ascontiguousarray(np.asarray(b_ih, f32)[None, 2 * H:])
    bhn = np.ascontiguousarray(np.asarray(b_hh, f32)[None, 2 * H:])
    btp = np.ascontiguousarray(np.asarray(b_tp, f32)[None, :])
    ones = np.ones((1, BL), f32)
    in_maps = []
    for c in range(NCORES):
        sl = slice(c * BL, (c + 1) * BL)
        x0c = np.ascontiguousarray(x0[sl], f32)
        h0c = np.ascontiguousarray(np.asarray(h, f32)[sl])
        in_maps.append({
            "x0": x0c,
            "x0T": np.ascontiguousarray(x0c.T),
            "h0": h0c,
            "h0T": np.ascontiguousarray(h0c.T),
            "W_ihT": W_ihT, "W_hhT": W_hhT, "W_tpT": W_tpT,
            "brz": brz, "bin": bin_, "bhn": bhn, "btp": btp,
            "ones": ones,
        })
    return in_maps


def kernel(h, gt, W_ih, W_hh, b_ih, b_hh, W_tp, b_tp, time_steps):
    from concourse.bass_utils import run_bass_kernel_spmd
    nsteps = int(time_steps)
    assert nsteps == T, f"kernel hardcodes T={T}, got {nsteps}"
    nc = _get_nc(nsteps)
    in_maps = make_in_maps(h, gt, W_ih, W_hh, b_ih, b_hh, W_tp, b_tp)
    res = run_bass_kernel_spmd(nc, in_maps, core_ids=list(range(NCORES)),
                               trace=False)
    Y = np.concatenate([res.results[c]["Y"] for c in range(NCORES)], axis=0)
    return Y.astype(np.float32)



# revision 3
# speedup vs baseline: 6.5488x; 6.5488x over previous
"""GRU decoder kernel for Trainium2 (Bass/Tile), data-parallel over 8 NeuronCores.

Problem: nn_Decoder (B=512, T=128, D=256, H=1024), PyTorch GRUCell semantics:
    gi = x @ W_ih.T + b_ih ; gh = h @ W_hh.T + b_hh
    r = sig(gi_r + gh_r); z = sig(gi_z + gh_z); n = tanh(gi_n + r*gh_n)
    h' = (1-z)*n + z*h ; y = x + h' @ W_tp.T + b_tp ; x' = y   (x0 = gt[:,0,:])

Sharding: batch 512 -> 64 per core, weights replicated.

Design (v2): weight-stationary fp16 matmuls on the full 128x128 PE array.
All state/gates live TRANSPOSED: [dim-chunk on 128 partitions, batch=64 free].
Per gate-chunk g (24 of them = r0..7, z0..7, n0..7), accumulate in PSUM:
    p[:, g] = diag(bias_g) @ ones + sum_c W.T[c-chunk, g-chunk] @ hq_c (+ x terms)
fp16 weight error is compensated for W_ih and W_tp by a second "lo" matmul:
W = W_hi + W_lo, with W_lo pre-scaled by 2^10 (avoids fp16 subnormals) and the
moving operand pre-scaled by 2^-10. W_hh needs no compensation (|h| <= 1).
Gate math runs on DVE/ACT in transposed space (z-tail split in halves to
pipeline against PE). h state is fp16 (hq); x state is fp32 (xT32) for the
residual chain. y(t) == x(t+1) transposed, so Y is DMA'd transposed as
[T, 2, 128, 64] and the host untransposes. No PE transposes anywhere.
~354 matmuls/step (mostly N=64 fp16 @ ~29 ns) ~= 10.5 us/step/core.
"""
import numpy as np

B, T, D, H = 512, 128, 256, 1024
NCORES = 8
BL = B // NCORES  # 64 batch rows per core
H3 = 3 * H
KH = H // 128     # 8 h chunks
KD = D // 128     # 2 x chunks
NG = H3 // 128    # 24 gate chunks (r:0-7, z:8-15, n:16-23)
LOSC = 1024.0     # W_lo scale factor (2^10)

_CACHE = {}


def _build(nsteps, loop_reps=None):
    import concourse.mybir as mybir
    import concourse.tile as tile
    from concourse import bacc

    F32 = mybir.dt.float32
    F16 = mybir.dt.float16
    AF = mybir.ActivationFunctionType

    nc = bacc.Bacc(None, target_bir_lowering=False)

    # --- DRAM I/O (per core). Host pre-transposes and pre-quantizes.
    h0T_d = nc.dram_tensor("h0T", [H, BL], F32, kind="ExternalInput")
    x0T_d = nc.dram_tensor("x0T", [D, BL], F32, kind="ExternalInput")
    whh_d = nc.dram_tensor("whh16", [H, H3], F16, kind="ExternalInput")
    wihh_d = nc.dram_tensor("wih_hi", [D, H3], F16, kind="ExternalInput")
    wihl_d = nc.dram_tensor("wih_lo", [D, H3], F16, kind="ExternalInput")
    wtph_d = nc.dram_tensor("wtp_hi", [H, D], F16, kind="ExternalInput")
    wtpl_d = nc.dram_tensor("wtp_lo", [H, D], F16, kind="ExternalInput")
    # 34 bias diagonal blocks: r(8) z(8) hn(8) in(8) tp(2), each [128,128]
    bdiag_d = nc.dram_tensor("bdiag", [128, 34 * 128], F16, kind="ExternalInput")
    ones_d = nc.dram_tensor("ones16", [128, BL], F16, kind="ExternalInput")
    Y_d = nc.dram_tensor("Y", [T, KD, 128, BL], F32, kind="ExternalOutput")

    with tile.TileContext(nc) as tc:
        with (
            tc.tile_pool(name="wpool", bufs=1) as wpool,
            tc.tile_pool(name="state", bufs=2) as state,
            tc.tile_pool(name="gates", bufs=2) as gates,
            tc.tile_pool(name="ps_r", bufs=2, space="PSUM") as ps_r,
            tc.tile_pool(name="ps_z", bufs=1, space="PSUM") as ps_z,
            tc.tile_pool(name="ps_hn", bufs=2, space="PSUM") as ps_hn,
            tc.tile_pool(name="ps_in", bufs=1, space="PSUM") as ps_in,
            tc.tile_pool(name="ps_y", bufs=2, space="PSUM") as ps_y,
        ):
            # --- weights resident in SBUF (fp16)
            whh = wpool.tile([128, KH, NG, 128], F16)     # W_hhT chunks
            for c in range(KH):
                nc.sync.dma_start(out=whh[:, c, :, :],
                                  in_=whh_d[c * 128:(c + 1) * 128, :])
            wih_hi = wpool.tile([128, KD, NG, 128], F16)
            wih_lo = wpool.tile([128, KD, NG, 128], F16)
            for c in range(KD):
                nc.sync.dma_start(out=wih_hi[:, c, :, :],
                                  in_=wihh_d[c * 128:(c + 1) * 128, :])
                nc.sync.dma_start(out=wih_lo[:, c, :, :],
                                  in_=wihl_d[c * 128:(c + 1) * 128, :])
            wtp_hi = wpool.tile([128, KH, KD, 128], F16)
            wtp_lo = wpool.tile([128, KH, KD, 128], F16)
            for c in range(KH):
                nc.sync.dma_start(out=wtp_hi[:, c, :, :],
                                  in_=wtph_d[c * 128:(c + 1) * 128, :])
                nc.sync.dma_start(out=wtp_lo[:, c, :, :],
                                  in_=wtpl_d[c * 128:(c + 1) * 128, :])
            bdiag = wpool.tile([128, 34, 128], F16)
            nc.sync.dma_start(out=bdiag, in_=bdiag_d[:, :])
            ones16 = wpool.tile([128, BL], F16)
            nc.sync.dma_start(out=ones16, in_=ones_d[:, :])

            # --- initial state (transposed): h fp16 (+scaled), x fp32 + fp16
            hT32_0 = state.tile([128, KH * BL], F32, tag="h32init")
            for c in range(KH):
                nc.sync.dma_start(out=hT32_0[:, c * BL:(c + 1) * BL],
                                  in_=h0T_d[c * 128:(c + 1) * 128, :])
            xT32 = state.tile([128, KD * BL], F32, tag="x32")
            for c in range(KD):
                nc.sync.dma_start(out=xT32[:, c * BL:(c + 1) * BL],
                                  in_=x0T_d[c * 128:(c + 1) * 128, :])
            hq = state.tile([128, KH * BL], F16, tag="hq")
            nc.scalar.copy(hq, hT32_0)
            hqs = state.tile([128, KH * BL], F16, tag="hqs")
            nc.scalar.activation(hqs, hT32_0, AF.Copy, scale=1.0 / LOSC)
            xq = state.tile([128, KD * BL], F16, tag="xq")
            nc.scalar.copy(xq, xT32)
            xqs = state.tile([128, KD * BL], F16, tag="xqs")
            nc.scalar.activation(xqs, xT32, AF.Copy, scale=1.0 / LOSC)

            # bias diag indices
            BR, BZ, BHN, BIN, BTP = 0, 8, 16, 24, 32

            def gate_group(out, g, bidx, use_h, use_x):
                """One accumulation group into out=[128,BL]: bias + chunks."""
                nc.tensor.matmul(out, bdiag[:, bidx, :], ones16,
                                 start=True, stop=False)
                movs = []
                if use_h:
                    for c in range(KH):
                        movs.append((whh[:, c, g, :], hq[:, c * BL:(c + 1) * BL]))
                if use_x:
                    for c in range(KD):
                        movs.append((wih_hi[:, c, g, :], xq[:, c * BL:(c + 1) * BL]))
                        movs.append((wih_lo[:, c, g, :], xqs[:, c * BL:(c + 1) * BL]))
                for i, (st, mv) in enumerate(movs):
                    nc.tensor.matmul(out, st, mv,
                                     start=False, stop=(i == len(movs) - 1))

            from contextlib import nullcontext
            loop_cm = tc.For_i(0, loop_reps, 1) if loop_reps else nullcontext()
            with loop_cm:
              for t in range(nsteps):
                # ---- gate matmuls (chunk-major groups)
                p_r = ps_r.tile([128, KH * BL], F32, tag="r")
                for j in range(KH):
                    gate_group(p_r[:, j * BL:(j + 1) * BL], j, BR + j, True, True)
                p_hn = ps_hn.tile([128, KH * BL], F32, tag="hn")
                for j in range(KH):
                    gate_group(p_hn[:, j * BL:(j + 1) * BL], 16 + j, BHN + j,
                               True, False)
                p_in = ps_in.tile([128, KH * BL], F32, tag="in")
                for j in range(KH):
                    gate_group(p_in[:, j * BL:(j + 1) * BL], 16 + j, BIN + j,
                               False, True)

                # ---- n-path (full width; overlaps z matmuls below)
                r = gates.tile([128, KH * BL], F32, tag="r")
                nc.scalar.activation(r, p_r, AF.Sigmoid)
                t1 = gates.tile([128, KH * BL], F32, tag="t1")
                nc.vector.tensor_mul(t1, r, p_hn)
                t2 = gates.tile([128, KH * BL], F32, tag="t2")
                nc.vector.tensor_add(t2, t1, p_in)
                n = gates.tile([128, KH * BL], F32, tag="n")
                nc.scalar.activation(n, t2, AF.Tanh)
                d = gates.tile([128, KH * BL], F32, tag="d")
                nc.vector.tensor_sub(d, hq, n)

                p_z = ps_z.tile([128, KH * BL], F32, tag="z")
                for j in range(KH):
                    gate_group(p_z[:, j * BL:(j + 1) * BL], 8 + j, BZ + j,
                               True, True)

                # ---- z tail, split in halves to pipeline with PE
                HB = KH * BL // 2
                hq_new = state.tile([128, KH * BL], F16, tag="hq")
                hqs_new = state.tile([128, KH * BL], F16, tag="hqs")
                for k in range(2):
                    sl = slice(k * HB, (k + 1) * HB)
                    zk = gates.tile([128, HB], F32, tag=f"z{k}")
                    nc.scalar.activation(zk, p_z[:, sl], AF.Sigmoid)
                    uk = gates.tile([128, HB], F32, tag=f"u{k}")
                    nc.vector.tensor_mul(uk, zk, d[:, sl])
                    nc.vector.tensor_add(hq_new[:, sl], n[:, sl], uk)
                    nc.scalar.activation(hqs_new[:, sl], hq_new[:, sl],
                                         AF.Copy, scale=1.0 / LOSC)

                # ---- y head: p_y[:, gd] = diag(btp_gd) + sum_c WtpT\' @ hq\'
                p_y = ps_y.tile([128, KD * BL], F32, tag="y")
                for gd in range(KD):
                    out = p_y[:, gd * BL:(gd + 1) * BL]
                    nc.tensor.matmul(out, bdiag[:, BTP + gd, :], ones16,
                                     start=True, stop=False)
                    for c in range(KH):
                        nc.tensor.matmul(out, wtp_hi[:, c, gd, :],
                                         hq_new[:, c * BL:(c + 1) * BL],
                                         start=False, stop=False)
                        nc.tensor.matmul(out, wtp_lo[:, c, gd, :],
                                         hqs_new[:, c * BL:(c + 1) * BL],
                                         start=False, stop=(c == KH - 1))

                # ---- x' = y = x + p_y (fp32 state); requantize; DMA out
                xT32_new = state.tile([128, KD * BL], F32, tag="x32")
                nc.vector.tensor_add(xT32_new, xT32, p_y)
                xq_new = state.tile([128, KD * BL], F16, tag="xq")
                nc.scalar.copy(xq_new, xT32_new)
                xqs_new = state.tile([128, KD * BL], F16, tag="xqs")
                nc.scalar.activation(xqs_new, xT32_new, AF.Copy, scale=1.0 / LOSC)
                for c in range(KD):
                    nc.sync.dma_start(out=Y_d[t % T, c, :, :],
                                      in_=xT32_new[:, c * BL:(c + 1) * BL])

                hq, hqs = hq_new, hqs_new
                xT32, xq, xqs = xT32_new, xq_new, xqs_new

    nc.finalize()
    return nc


def _get_nc(nsteps):
    if nsteps not in _CACHE:
        _CACHE[nsteps] = _build(nsteps)
    return _CACHE[nsteps]


def make_in_maps(h, gt, W_ih, W_hh, b_ih, b_hh, W_tp, b_tp):
    """Host-side prep: slice batch per core, transpose + fp16 hi/lo split."""
    f32, f16 = np.float32, np.float16

    def hilo(W):
        Whi = np.asarray(W, f32).astype(f16)
        Wlo = ((np.asarray(W, f32) - Whi.astype(f32)) * LOSC).astype(f16)
        return Whi, Wlo

    W_ihT = np.ascontiguousarray(np.asarray(W_ih, f32).T)     # [D, 3H]
    W_hhT = np.ascontiguousarray(np.asarray(W_hh, f32).T)     # [H, 3H]
    W_tpT = np.ascontiguousarray(np.asarray(W_tp, f32).T)     # [H, D]
    whh16 = W_hhT.astype(f16)
    wih_hi, wih_lo = hilo(W_ihT)
    wtp_hi, wtp_lo = hilo(W_tpT)

    b_sum = (np.asarray(b_ih, f32) + np.asarray(b_hh, f32))
    bdiag = np.zeros((128, 34 * 128), f32)
    for j in range(8):   # r
        np.fill_diagonal(bdiag[:, j * 128:(j + 1) * 128],
                         b_sum[j * 128:(j + 1) * 128])
    for j in range(8):   # z
        np.fill_diagonal(bdiag[:, (8 + j) * 128:(9 + j) * 128],
                         b_sum[H + j * 128:H + (j + 1) * 128])
    bhh_n = np.asarray(b_hh, f32)[2 * H:]
    bih_n = np.asarray(b_ih, f32)[2 * H:]
    for j in range(8):   # hn
        np.fill_diagonal(bdiag[:, (16 + j) * 128:(17 + j) * 128],
                         bhh_n[j * 128:(j + 1) * 128])
    for j in range(8):   # in
        np.fill_diagonal(bdiag[:, (24 + j) * 128:(25 + j) * 128],
                         bih_n[j * 128:(j + 1) * 128])
    btp = np.asarray(b_tp, f32)
    for j in range(KD):  # tp
        np.fill_diagonal(bdiag[:, (32 + j) * 128:(33 + j) * 128],
                         btp[j * 128:(j + 1) * 128])
    bdiag = bdiag.astype(f16)
    ones16 = np.ones((128, BL), f16)

    x0 = np.ascontiguousarray(np.asarray(gt, f32)[:, 0, :])   # [B, D]
    h0 = np.asarray(h, f32)
    in_maps = []
    for core in range(NCORES):
        sl = slice(core * BL, (core + 1) * BL)
        in_maps.append({
            "h0T": np.ascontiguousarray(h0[sl].T),
            "x0T": np.ascontiguousarray(x0[sl].T),
            "whh16": whh16,
            "wih_hi": wih_hi, "wih_lo": wih_lo,
            "wtp_hi": wtp_hi, "wtp_lo": wtp_lo,
            "bdiag": bdiag, "ones16": ones16,
        })
    return in_maps


def kernel(h, gt, W_ih, W_hh, b_ih, b_hh, W_tp, b_tp, time_steps):
    from concourse.bass_utils import run_bass_kernel_spmd
    nsteps = int(time_steps)
    assert nsteps == T, f"kernel hardcodes T={T}, got {nsteps}"
    nc = _get_nc(nsteps)
    in_maps = make_in_maps(h, gt, W_ih, W_hh, b_ih, b_hh, W_tp, b_tp)
    res = run_bass_kernel_spmd(nc, in_maps, core_ids=list(range(NCORES)),
                               trace=False)
    # Y per core: [T, KD, 128, BL] transposed -> [BL, T, D]
    outs = []
    for c in range(NCORES):
        Yt = res.results[c]["Y"].reshape(T, D, BL)     # [T, D, BL]
        outs.append(np.ascontiguousarray(Yt.transpose(2, 0, 1)))
    return np.concatenate(outs, axis=0).astype(np.float32)
